# revision 37
# baseline (speedup 1.0000x reference)
"""2-layer GAT + MLP head on 8 TRN2 NeuronCores.

Strategy (dst-sharding, one SPMD program):
- Nodes padded to NP=20480; each core owns 2560 dst nodes = 20 tiles of 128.
- Edges (incl. self-loops, PyG mean-fill edge attr) sorted by dst, grouped
  into 128-dst tiles. Tiles are load-balanced across SPMD "slots": each core
  processes its k-th largest tile at slot k, so the shared per-slot chunk
  count K_r = max over cores is tight.
- Layer tables in HBM, row = [h bf16 x512 | alpha_src bf16 x4 | pad] (1056B).
  alpha_dst never goes through the table: each core keeps its own dst tiles'
  alpha_dst in SBUF (computed in phase A / fin0).
- H0 uses a per-core "own-first" row layout (own dst tiles at rows 0:2560) so
  phase A can capture alpha_dst for the core's own tiles with static control
  flow. H1 uses the AllGather-imposed (group, core, loc) layout.
- Aggregation per 128-edge chunk: dma_gather rows by src; p = exp(leakyrelu(
  asrc+adst+aedge)) in bf16; out[dst] += one-hot matmul (PSUM); softmax
  denominator via a second matmul with rhs=p. alpha_dst expanded per edge by
  a small matmul with a bf16 one-hot-transpose (oht) stationary.
- Layer-1 linear runs per dst tile right after its layer-0 finalize; table
  groups are AllGathered (Shared output) as soon as ready.
"""

import numpy as np
import ml_dtypes

import concourse.bacc as bacc
import concourse.bass as bass
import concourse.mybir as mybir
import concourse.tile as tile
from concourse.bass_utils import run_bass_kernel_spmd

F32 = mybir.dt.float32
BF16 = mybir.dt.bfloat16
I16 = mybir.dt.int16
AF = mybir.ActivationFunctionType
OP = mybir.AluOpType

NCORES = 8
SCC = 16  # chunks (of 128 edges) per gather super-chunk


def _bcast4(ap_tile, j, reps):
    """[128, SCC, 4] tile -> [128, 4, reps] zero-step broadcast AP of slot j."""
    sl = ap_tile[:, j, :]
    return bass.AP(sl.tensor, sl.offset, [list(sl.ap[0]), list(sl.ap[-1]), [0, reps]])


def _bcast_free(ap2d, reps):
    """[128, k] tile -> [128, k, reps] zero-step broadcast AP."""
    sl = ap2d[:, :]
    return bass.AP(sl.tensor, sl.offset, [list(sl.ap[0]), list(sl.ap[-1]), [0, reps]])


def _build_program(NP, F_IN, HC, H, C, NT, K_t, FTS, NAG,
                   use_b0, use_b1, use_l0b, use_l1b):
    NCHUNK = int(sum(K_t))
    E_pad = NCHUNK * 128
    SW = E_pad // 16
    TW = HC + 128  # bf16 table row: h | asrc bf16 x4 | pad (256B-mult rows)
    KB = HC // 128
    GPG = NT // NAG          # dst tiles per allgather group
    GR = GPG * 128           # rows per group per core

    nc = bacc.Bacc(dynamic_dma_scratch_size=65536, num_swdge_queues=4)
    P = nc.declare_dram_parameter

    xT = P("xT", [F_IN, NP], BF16, isOutput=False)
    r0h = P("r0h", [F_IN, HC], BF16, isOutput=False)
    r0a = P("r0a", [F_IN, 8], BF16, isOutput=False)
    r1h = P("r1h", [HC, HC], BF16, isOutput=False)
    r1a = P("r1a", [HC, 8], BF16, isOutput=False)
    r2 = P("r2", [HC, FTS], BF16, isOutput=False)
    r3 = P("r3", [FTS, 1], BF16, isOutput=False)
    b0t = P("b0t", [128, HC], F32, isOutput=False)
    b1t = P("b1t", [128, HC], F32, isOutput=False)
    l0bt = P("l0bt", [128, FTS], F32, isOutput=False)
    l1bt = P("l1bt", [128, 1], F32, isOutput=False)
    identb = P("identb", [128, 128], BF16, isOutput=False)
    srcw0 = P("srcw0", [128, SW], I16, isOutput=False)
    srcw1 = P("srcw1", [128, SW], I16, isOutput=False)
    ohb = P("ohb", [128, NCHUNK * 128], BF16, isOutput=False)
    oht = P("oht", [128, NCHUNK * 128], BF16, isOutput=False)
    ae0 = P("ae0", [128, NCHUNK, 4], BF16, isOutput=False)
    ae1 = P("ae1", [128, NCHUNK, 4], BF16, isOutput=False)
    outp = P("out", [NT * 128, 1], F32, isOutput=True)

    with tile.TileContext(nc) as tc:
        with (
            tc.tile_pool(name="const", bufs=1) as const,
            tc.tile_pool(name="stage", bufs=2) as stage,
            tc.tile_pool(name="work", bufs=3) as work,
            tc.tile_pool(name="tp", bufs=6) as tp,
            tc.tile_pool(name="adp", bufs=1) as adp,
            tc.tile_pool(name="psacc", bufs=2, space="PSUM") as psacc,
            tc.tile_pool(name="pss", bufs=2, space="PSUM") as pss,
            tc.tile_pool(name="pstr", bufs=2, space="PSUM") as pstr,
            tc.tile_pool(name="dram", bufs=1, space="DRAM") as dram,
        ):
            TWC = HC + 8  # useful row prefix: h | asrc (1040B)
            H0 = dram.tile([NP, TW], BF16, tag="H0")
            H1 = dram.tile([NP, TW], BF16, tag="H1")
            H1g = [dram.tile([GR, TW], BF16, tag=f"H1g{g}", name=f"H1g{g}")
                   for g in range(NAG)]

            _cn = [0]

            def cload(ap_in, shape, dt=F32, tag=None):
                _cn[0] += 1
                cname = tag or f"c{_cn[0]}"
                t = const.tile(shape, dt, tag=cname, name=f"{cname}_{_cn[0]}")
                nc.sync.dma_start(out=t[:], in_=ap_in)
                return t

            r0h_s = cload(r0h[:, :], [F_IN, HC], BF16)
            r0a_s = cload(r0a[:, :], [F_IN, 8], BF16)
            r1h_s = [cload(r1h[k * 128:(k + 1) * 128, :], [128, HC], BF16)
                     for k in range(KB)]
            r1a_s = [cload(r1a[k * 128:(k + 1) * 128, :], [128, 8], BF16)
                     for k in range(KB)]
            r2_s = [cload(r2[k * 128:(k + 1) * 128, :], [128, FTS], BF16)
                    for k in range(KB)]
            r3_s = cload(r3[:, :], [FTS, 1], BF16)
            id_s = cload(identb[:, :], [128, 128], BF16)
            srcw0_s = cload(srcw0[:, :], [128, SW], I16)
            srcw1_s = cload(srcw1[:, :], [128, SW], I16)
            ae0_s = cload(ae0[:, :, :], [128, NCHUNK, 4], BF16, tag="ae0")
            ae1_s = cload(ae1[:, :, :], [128, NCHUNK, 4], BF16, tag="ae1")
            b0_s = cload(b0t[:, :], [128, HC]) if use_b0 else None
            b1_s = cload(b1t[:, :], [128, HC]) if use_b1 else None
            l0b_s = cload(l0bt[:, :], [128, FTS]) if use_l0b else None
            l1b_s = cload(l1bt[:, :], [128, 1]) if use_l1b else None

            adt0 = {}
            adt1 = {}

            # ---- phase A: layer-0 table (xT comes pre-permuted: own-first)
            MT = NP // 128
            XB = 4  # x tiles per write block
            nc._state.push_named_scope("phaseA")
            XL = 16  # x tiles per load
            lx = None
            for blk in range(MT // XB):
                if blk % (XL // XB) == 0:
                    lx = stage.tile([F_IN, XL * 128], BF16, tag="lx", bufs=2)
                    nc.sync.dma_start(
                        out=lx[:],
                        in_=xT[:, (blk // (XL // XB)) * XL * 128:
                               (blk // (XL // XB) + 1) * XL * 128])
                lo = (blk % (XL // XB)) * XB
                stw = stage.tile([128, XB, TWC], BF16, tag="hblk", bufs=2)
                for j in range(XB):
                    mr = blk * XB + j
                    lj = lo + j
                    ph = psacc.tile([128, HC], F32, tag="ph")
                    nc.tensor.matmul(ph[:], lx[:, lj * 128:(lj + 1) * 128],
                                     r0h_s[:], start=True, stop=True)
                    pa = pss.tile([128, 8], F32, tag="pss")
                    nc.tensor.matmul(pa[:], lx[:, lj * 128:(lj + 1) * 128],
                                     r0a_s[:], start=True, stop=True)
                    if mr % 2 == 0:
                        nc.vector.tensor_copy(stw[:, j, 0:HC], ph[:])
                    else:
                        nc.scalar.activation(stw[:, j, 0:HC], ph[:], AF.Copy)
                    nc.vector.tensor_copy(stw[:, j, HC:HC + 4], pa[:, 0:4])
                    if mr < NT:
                        ad = adp.tile([128, 4], BF16, tag=f"adt0_{mr}",
                                      name=f"adt0_{mr}")
                        nc.vector.tensor_copy(ad[:], pa[:, 4:8])
                        adt0[mr] = ad
                # one blocked write: H0 rows [blk*XB*128, (blk+1)*XB*128)
                base = blk * XB * 128 * TW
                h0blk = bass.AP(H0.tensor, base,
                                [[TW, 128], [128 * TW, XB], [1, TWC]])
                nc.sync.dma_start(out=h0blk, in_=stw[:])
            nc._state.pop_named_scope("phaseA")

            # ---- aggregation over one layer's edges ----
            def agg_layer(tbl, srcw_s, ae_s, adts, finalize):
                # chunk q -> owning dst slot
                t_of_q = []
                for t in range(NT):
                    t_of_q += [t] * K_t[t]
                NSUP = -(-NCHUNK // SCC)

                def ensure_super(s):
                    cnt = min(SCC * 128, E_pad - s * SCC * 128)
                    nch = cnt // 128
                    gA = stage.tile([128, SCC, TW], BF16, tag="gA",
                                    name="gA", bufs=4)
                    c0 = s * SCC * 8
                    qparts = []
                    base = 0
                    nq = min(4, nch)
                    for qi in range(nq):
                        take = (nch - base + (nq - qi) - 1) // (nq - qi)
                        qparts.append((base, take, qi))
                        base += take
                    for (b0, tk, qi) in qparts:
                        nc.gpsimd.dma_gather(
                            gA[:, b0:b0 + tk, :], tbl[:, :],
                            srcw_s[:, c0 + b0 * 8:c0 + (b0 + tk) * 8],
                            tk * 128, tk * 128, TW,
                            single_packet=False, queue_num=qi)
                    ohb_t = stage.tile([128, SCC * 128], BF16, tag="ohb",
                                       name="ohb_t", bufs=3)
                    nc.sync.dma_start(
                        out=ohb_t[:, 0:nch * 128],
                        in_=ohb[:, s * SCC * 128:s * SCC * 128 + nch * 128])
                    oht_t = stage.tile([128, SCC * 128], BF16, tag="oht",
                                       name="oht_t", bufs=2)
                    nc.sync.dma_start(
                        out=oht_t[:, 0:nch * 128],
                        in_=oht[:, s * SCC * 128:s * SCC * 128 + nch * 128])
                    # expand alpha_dst per edge via ohT matmul; stage to SBUF
                    # so the PSUM bank frees fast and peads can run far ahead
                    pead = pstr.tile([128, SCC * 4], F32, tag="pt", name="pead")
                    for jj in range(nch):
                        qq = s * SCC + jj
                        nc.tensor.matmul(
                            pead[:, jj * 4:(jj + 1) * 4],
                            oht_t[:, jj * 128:(jj + 1) * 128],
                            adts[t_of_q[qq]][:],
                            start=True, stop=True)
                    peads = work.tile([128, SCC * 4], F32, tag="peads", bufs=6)
                    nc.vector.tensor_copy(peads[:, 0:nch * 4],
                                          pead[:, 0:nch * 4])
                    t0 = work.tile([128, SCC, 4], BF16, tag="t0", bufs=2)
                    nc.vector.tensor_add(
                        t0[:, 0:nch, :], gA[:, 0:nch, HC:HC + 4],
                        ae_s[:, s * SCC:s * SCC + nch, :])
                    t1 = work.tile([128, SCC, 4], F32, tag="t1", bufs=2)
                    nc.vector.tensor_add(
                        t1[:, 0:nch, :], t0[:, 0:nch, :],
                        peads[:, 0:nch * 4].rearrange("x (a b) -> x a b", b=4))
                    t2 = work.tile([128, SCC, 4], F32, tag="t2", bufs=2)
                    nc.scalar.activation(
                        t2[:, 0:nch, :], t1[:, 0:nch, :], AF.Copy, scale=0.2)
                    t3 = work.tile([128, SCC, 4], F32, tag="t3", bufs=2)
                    nc.vector.tensor_max(
                        t3[:, 0:nch, :], t1[:, 0:nch, :], t2[:, 0:nch, :])
                    p = work.tile([128, SCC, 4], F32, tag="p", bufs=2)
                    nc.scalar.activation(p[:, 0:nch, :], t3[:, 0:nch, :], AF.Exp)
                    pb = work.tile([128, SCC, 4], BF16, tag="pb", bufs=2)
                    nc.scalar.activation(pb[:, 0:nch, :], p[:, 0:nch, :], AF.Copy)
                    return gA, ohb_t, p, pb

                q = 0
                gA = ohb_t = p = pb = None
                for t in range(NT):
                    ps_o = psacc.tile([128, HC], F32, tag="ph")
                    ps_s = pss.tile([128, 8], F32, tag="pss")
                    for k in range(K_t[t]):
                        s, j = divmod(q, SCC)
                        if j == 0:
                            gA, ohb_t, p, pb = ensure_super(s)
                        gp = work.tile([128, HC], BF16, tag="gp", bufs=4)
                        H2 = H // 2
                        sl = p[:, j, 0:H2]
                        bc2 = bass.AP(sl.tensor, sl.offset,
                                      [list(sl.ap[0]), list(sl.ap[-1]),
                                       [0, C]])
                        nc.vector.tensor_mul(
                            gp[:, 0:H2 * C].rearrange("x (h c) -> x h c", h=H2),
                            gA[:, j, 0:H2 * C].rearrange(
                                "x (h c) -> x h c", h=H2),
                            bc2)
                        for h in range(H2, H):
                            nc.scalar.activation(
                                gp[:, h * C:(h + 1) * C],
                                gA[:, j, h * C:(h + 1) * C],
                                AF.Copy, scale=p[:, j, h:h + 1])
                        first, last = (k == 0), (k == K_t[t] - 1)
                        oh_j = ohb_t[:, j * 128:(j + 1) * 128]
                        nc.tensor.matmul(ps_o[:], oh_j, gp[:],
                                         start=first, stop=last)
                        nc.tensor.matmul(ps_s[:, 0:4], oh_j, pb[:, j, :],
                                         start=first, stop=last)
                        q += 1
                    finalize(t, ps_o, ps_s)

            def norm_relu(ps_o, ps_s, bias_s):
                sp = work.tile([128, 4], F32, tag="sp")
                nc.vector.tensor_scalar_add(sp[:], ps_s[:, 0:4], 1e-16)
                rc = work.tile([128, 4], F32, tag="rc")
                nc.vector.reciprocal(rc[:], sp[:])
                ar = work.tile([128, HC], BF16, tag="ar", bufs=2)
                if bias_s is None:
                    # relu(x*c) == c*relu(x) for c>0: fused PSUM->bf16
                    for h in range(H):
                        nc.scalar.activation(
                            ar[:, h * C:(h + 1) * C], ps_o[:, h * C:(h + 1) * C],
                            AF.Relu, scale=rc[:, h:h + 1])
                else:
                    ao = work.tile([128, HC], F32, tag="ao", bufs=2)
                    nc.vector.tensor_mul(
                        ao[:].rearrange("x (h c) -> x h c", h=H),
                        ps_o[:].rearrange("x (h c) -> x h c", h=H),
                        _bcast_free(rc, C))
                    ab = work.tile([128, HC], F32, tag="ao", bufs=2)
                    nc.vector.tensor_add(ab[:], ao[:], bias_s[:])
                    nc.scalar.activation(ar[:], ab[:], AF.Relu)
                return ar

            # ---- layer-0 finalize: transpose + layer-1 linear + H1 group AG
            def fin0(t, ps_o, ps_s):
                ar = norm_relu(ps_o, ps_s, b0_s)
                a0k = []
                for kk in range(KB):
                    pt = pstr.tile([128, 128], BF16, tag="ptb")
                    nc.tensor.transpose(pt[:], ar[:, kk * 128:(kk + 1) * 128],
                                        id_s[:])
                    ak = tp.tile([128, 128], BF16, tag="a1T", name=f"a0k{kk}")
                    nc.vector.tensor_copy(ak[:], pt[:])
                    a0k.append(ak)
                ph1 = psacc.tile([128, HC], F32, tag="ph")
                pa1 = pss.tile([128, 8], F32, tag="pss")
                for kk in range(KB):
                    first, last = (kk == 0), (kk == KB - 1)
                    nc.tensor.matmul(ph1[:], a0k[kk][:], r1h_s[kk][:],
                                     start=first, stop=last)
                    nc.tensor.matmul(pa1[:], a0k[kk][:], r1a_s[kk][:],
                                     start=first, stop=last)
                st = stage.tile([128, TWC], BF16, tag="hrow", bufs=2)
                if t % 2 == 0:
                    nc.vector.tensor_copy(st[:, 0:HC], ph1[:])
                else:
                    nc.scalar.activation(st[:, 0:HC], ph1[:], AF.Copy)
                nc.vector.tensor_copy(st[:, HC:HC + 4], pa1[:, 0:4])
                ad = adp.tile([128, 4], BF16, tag=f"adt1_{t}", name=f"adt1_{t}")
                nc.vector.tensor_copy(ad[:], pa1[:, 4:8])
                adt1[t] = ad
                g = t // GPG
                loc = t % GPG
                nc.sync.dma_start(
                    out=H1g[g][loc * 128:(loc + 1) * 128, 0:TWC], in_=st[:])
                if loc == GPG - 1:
                    nc.gpsimd.collective_compute(
                        "AllGather", OP.bypass,
                        replica_groups=[list(range(NCORES))],
                        ins=[H1g[g].opt()],
                        outs=[H1[g * NCORES * GR:(g + 1) * NCORES * GR,
                                 :].opt()],
                    )

            nc._state.push_named_scope("phaseB")
            agg_layer(H0, srcw0_s, ae0_s, adt0, fin0)
            nc._state.pop_named_scope("phaseB")

            # ---- layer-1 aggregation + MLP head per dst tile ----
            def fin1(t, ps_o, ps_s):
                ar = norm_relu(ps_o, ps_s, b1_s)
                h2p = psacc.tile([128, FTS], F32, tag="ph")
                for kk in range(KB):
                    pt = pstr.tile([128, 128], BF16, tag="ptb")
                    nc.tensor.transpose(pt[:], ar[:, kk * 128:(kk + 1) * 128],
                                        id_s[:])
                    a1k = tp.tile([128, 128], BF16, tag="a1T")
                    nc.vector.tensor_copy(a1k[:], pt[:])
                    nc.tensor.matmul(h2p[:], a1k[:], r2_s[kk][:],
                                     start=(kk == 0), stop=(kk == KB - 1))
                if use_l0b:
                    h2b = work.tile([128, FTS], F32, tag="h2b")
                    nc.vector.tensor_add(h2b[:], h2p[:], l0b_s[:])
                else:
                    h2b = h2p
                h2r = work.tile([128, FTS], BF16, tag="h2r")
                nc.scalar.activation(h2r[:], h2b[:], AF.Relu)
                pt2 = pstr.tile([128, 128], BF16, tag="ptb")
                nc.tensor.transpose(pt2[:], h2r[:], id_s[:])
                h2T = tp.tile([128, 128], BF16, tag="a1T")
                nc.vector.tensor_copy(h2T[:], pt2[:])
                po = pss.tile([128, 8], F32, tag="pss")
                nc.tensor.matmul(po[:, 0:1], h2T[:], r3_s[:],
                                 start=True, stop=True)
                ob = work.tile([128, 1], F32, tag="ob")
                if use_l1b:
                    nc.vector.tensor_add(ob[:], po[:, 0:1], l1b_s[:])
                else:
                    nc.vector.tensor_copy(ob[:], po[:, 0:1])
                nc.sync.dma_start(out=outp[t * 128:(t + 1) * 128, :],
                                  in_=ob[:])

            nc._state.push_named_scope("phaseD")
            agg_layer(H1, srcw1_s, ae1_s, adt1, fin1)
            nc._state.pop_named_scope("phaseD")

    nc.finalize()
    return nc


def _wrap_idx(v, E_pad):
    blk = np.zeros((16, E_pad // 16), np.int16)
    ar = np.arange(E_pad)
    blk[ar % 16, ar // 16] = v.astype(np.int16)
    return np.tile(blk, (8, 1))


def kernel(x, edge_index, edge_weights,
           W0, as0, ad0, We0, ae0, b0,
           W1, as1, ad1, We1, ae1, b1,
           L0W, L0b, L1W, L1b):
    x = np.asarray(x, np.float32)
    N, F_IN = x.shape
    HC = W0.shape[0]
    H, C = np.asarray(as0).shape
    FTS = np.asarray(L0W).shape[0]

    NT = -(-N // (128 * NCORES))
    SHARD = NT * 128
    NP = SHARD * NCORES
    NAG = 1
    for cand in (10, 5, 4, 2):
        if NT % cand == 0:
            NAG = cand
            break
    GPG = NT // NAG
    GR = GPG * 128

    # ---- edges ----
    ew_in = np.asarray(edge_weights, np.float32)
    src = np.concatenate([np.asarray(edge_index[0]), np.arange(N)])
    dst = np.concatenate([np.asarray(edge_index[1]), np.arange(N)])
    ew = np.concatenate([ew_in, np.full(N, ew_in.mean(), np.float32)])
    order = np.argsort(dst, kind="stable")
    src_s, dst_s, ew_s = src[order], dst[order], ew[order]

    NTG = NP // 128
    tile_of = (dst_s // 128).astype(np.int64)
    tcounts = np.bincount(tile_of, minlength=NTG)
    tstart = np.concatenate([[0], np.cumsum(tcounts)])

    # ---- slot load balancing: slot r = each core's r-th largest tile ----
    cmat = tcounts.reshape(NCORES, NT)
    perm = np.argsort(-cmat, axis=1, kind="stable")   # [core, slot] -> tile
    slot_of = np.empty_like(perm)
    for i in range(NCORES):
        slot_of[i, perm[i]] = np.arange(NT)
    K_t = [max(1, int(max(-(-cmat[i, perm[i, r]] // 128)
                          for i in range(NCORES))))
           for r in range(NT)]
    NCHUNK = int(sum(K_t))
    E_pad = NCHUNK * 128

    # ---- node -> H1 table row (AllGather layout: group, core, loc) ----
    nodes = np.arange(NP)
    core = nodes // SHARD
    rr = nodes % SHARD
    tloc = rr // 128
    off = rr % 128
    slot_n = slot_of[core, tloc]
    gg = slot_n // GPG
    loc = slot_n % GPG
    t1row_of_n = gg * (NCORES * GR) + core * GR + loc * 128 + off

    # ---- weight folding (host, O(weights)) ----
    as0 = np.asarray(as0, np.float32)
    ad0 = np.asarray(ad0, np.float32)
    ae0w = np.asarray(ae0, np.float32)
    as1 = np.asarray(as1, np.float32)
    ad1 = np.asarray(ad1, np.float32)
    ae1w = np.asarray(ae1, np.float32)
    W0 = np.asarray(W0, np.float32)
    W1 = np.asarray(W1, np.float32)
    We0 = np.asarray(We0, np.float32)
    We1 = np.asarray(We1, np.float32)

    k0 = (We0.reshape(H, C) * ae0w).sum(1).astype(np.float32)
    k1 = (We1.reshape(H, C) * ae1w).sum(1).astype(np.float32)

    def fold(W, a):
        blk = np.zeros((HC, H), np.float32)
        for h in range(H):
            blk[h * C:(h + 1) * C, h] = a[h]
        return (W.T @ blk).astype(np.float32)

    bf = ml_dtypes.bfloat16
    r0h = W0.T.astype(bf)
    r0a = np.concatenate([fold(W0, as0), fold(W0, ad0)], 1).astype(bf)
    r1h = W1.T.astype(bf)
    r1a = np.concatenate([fold(W1, as1), fold(W1, ad1)], 1).astype(bf)
    r2 = np.asarray(L0W, np.float32).T.astype(bf)
    r3 = np.asarray(L1W, np.float32).T.astype(bf)

    b0t = np.tile(np.asarray(b0, np.float32)[None, :], (128, 1))
    b1t = np.tile(np.asarray(b1, np.float32)[None, :], (128, 1))
    l0bt = np.tile(np.asarray(L0b, np.float32)[None, :], (128, 1))
    l1bt = np.tile(np.asarray(L1b, np.float32).reshape(1, 1), (128, 1))
    identb = np.eye(128, dtype=np.float32).astype(bf)

    xa = np.zeros((NP, F_IN), np.float32)
    xa[:N] = x

    in_maps = []
    for i in range(NCORES):
        # per-core own-first H0 row layout
        own = np.empty(SHARD, np.int64)        # H0 row r*128+o -> node
        for r in range(NT):
            t = perm[i, r]
            own[r * 128:(r + 1) * 128] = i * SHARD + t * 128 + np.arange(128)
        is_own = core == i
        foreign = nodes[~is_own][np.argsort(t1row_of_n[~is_own], kind="stable")]
        row0_inv = np.concatenate([own, foreign])   # H0 row -> node
        row0_of = np.empty(NP, np.int64)
        row0_of[row0_inv] = nodes                   # node -> H0 row

        xT0 = np.ascontiguousarray(xa[row0_inv].T).astype(bf)

        src0 = np.zeros(E_pad, np.int64)
        src1 = np.zeros(E_pad, np.int64)
        dlocp = np.full(E_pad, -1, np.int64)
        ewp = np.zeros(E_pad, np.float32)
        offq = 0
        for r in range(NT):
            t = perm[i, r]
            g = i * NT + t
            cnt = int(tcounts[g])
            sl = slice(tstart[g], tstart[g] + cnt)
            src0[offq:offq + cnt] = row0_of[src_s[sl]]
            src1[offq:offq + cnt] = t1row_of_n[src_s[sl]]
            dlocp[offq:offq + cnt] = dst_s[sl] - (i * SHARD + t * 128)
            ewp[offq:offq + cnt] = ew_s[sl]
            # sort each 128-edge chunk by src row for gather locality
            for cb in range(offq, offq + cnt, 128):
                ce = min(cb + 128, offq + cnt)
                so = np.argsort(src1[cb:ce], kind="stable")
                src0[cb:ce] = src0[cb:ce][so]
                src1[cb:ce] = src1[cb:ce][so]
                dlocp[cb:ce] = dlocp[cb:ce][so]
                ewp[cb:ce] = ewp[cb:ce][so]
            offq += K_t[r] * 128
        ae0p = (ewp[:, None] * k0[None, :]).reshape(NCHUNK, 128, 4)
        ae0p = ae0p.transpose(1, 0, 2)
        ae1p = (ewp[:, None] * k1[None, :]).reshape(NCHUNK, 128, 4)
        ae1p = ae1p.transpose(1, 0, 2)
        # one-hot blocks: ohb[e, q*128 + d] / oht[d, q*128 + e]
        ohcube = np.zeros((NCHUNK, 128, 128), np.float32)  # [q, e, d]
        dl2 = dlocp.reshape(NCHUNK, 128)
        valid = dl2 >= 0
        qs, es = np.nonzero(valid)
        ohcube[qs, es, dl2[qs, es]] = 1.0
        ohb_np = np.ascontiguousarray(
            ohcube.transpose(1, 0, 2).reshape(128, NCHUNK * 128)).astype(bf)
        oht_np = np.ascontiguousarray(
            ohcube.transpose(2, 0, 1).reshape(128, NCHUNK * 128)).astype(bf)
        in_maps.append({
            "xT": xT0, "r0h": r0h, "r0a": r0a, "r1h": r1h, "r1a": r1a,
            "r2": r2, "r3": r3, "b0t": b0t, "b1t": b1t, "l0bt": l0bt,
            "l1bt": l1bt, "identb": identb,
            "srcw0": _wrap_idx(src0, E_pad), "srcw1": _wrap_idx(src1, E_pad),
            "ohb": ohb_np, "oht": oht_np,
            "ae0": np.ascontiguousarray(ae0p).astype(bf),
            "ae1": np.ascontiguousarray(ae1p).astype(bf),
        })

    nc = _build_program(NP, F_IN, HC, H, C, NT, K_t, FTS, NAG,
                        bool(np.any(b0)), bool(np.any(b1)),
                        bool(np.any(np.asarray(L0b))),
                        bool(np.any(np.asarray(L1b))))
    res = run_bass_kernel_spmd(nc, in_maps, list(range(NCORES)))
    out = np.empty(NP, np.float32)
    for i in range(NCORES):
        oi = res.results[i]["out"][:, 0]
        for t in range(NT):
            r = slot_of[i, t]
            out[i * SHARD + t * 128:i * SHARD + (t + 1) * 128] = \
                oi[r * 128:(r + 1) * 128]
    return out[:N].astype(np.float32)


# revision 40
# speedup vs baseline: 1.1202x; 1.1202x over previous
"""2-layer GAT + MLP head on 8 TRN2 NeuronCores.

Strategy (dst-sharding, one SPMD program):
- Nodes padded to NP=20480; each core owns 2560 dst nodes = 20 tiles of 128.
- Edges (incl. self-loops, PyG mean-fill edge attr) sorted by dst, grouped
  into 128-dst tiles. Tiles are load-balanced across SPMD "slots": each core
  processes its k-th largest tile at slot k, so the shared per-slot chunk
  count K_r = max over cores is tight.
- Layer tables in HBM, row = [h bf16 x512 | alpha_src bf16 x4 | pad] (1056B).
  alpha_dst never goes through the table: each core keeps its own dst tiles'
  alpha_dst in SBUF (computed in phase A / fin0).
- H0 uses a per-core "own-first" row layout (own dst tiles at rows 0:2560) so
  phase A can capture alpha_dst for the core's own tiles with static control
  flow. H1 uses the AllGather-imposed (group, core, loc) layout.
- Aggregation per 128-edge chunk: dma_gather rows by src; p = exp(leakyrelu(
  asrc+adst+aedge)) in bf16; out[dst] += one-hot matmul (PSUM); softmax
  denominator via a second matmul with rhs=p. alpha_dst expanded per edge by
  a small matmul with a bf16 one-hot-transpose (oht) stationary.
- Layer-1 linear runs per dst tile right after its layer-0 finalize; table
  groups are AllGathered (Shared output) as soon as ready.
"""

import numpy as np
import ml_dtypes

import concourse.bacc as bacc
import concourse.bass as bass
import concourse.mybir as mybir
import concourse.tile as tile
from concourse.bass_utils import run_bass_kernel_spmd

F32 = mybir.dt.float32
BF16 = mybir.dt.bfloat16
I16 = mybir.dt.int16
AF = mybir.ActivationFunctionType
OP = mybir.AluOpType

NCORES = 8
SCC = 16  # chunks (of 128 edges) per gather super-chunk


def _bcast4(ap_tile, j, reps):
    """[128, SCC, 4] tile -> [128, 4, reps] zero-step broadcast AP of slot j."""
    sl = ap_tile[:, j, :]
    return bass.AP(sl.tensor, sl.offset, [list(sl.ap[0]), list(sl.ap[-1]), [0, reps]])


def _bcast_free(ap2d, reps):
    """[128, k] tile -> [128, k, reps] zero-step broadcast AP."""
    sl = ap2d[:, :]
    return bass.AP(sl.tensor, sl.offset, [list(sl.ap[0]), list(sl.ap[-1]), [0, reps]])


def _build_program(NP, F_IN, HC, H, C, NT, K_t, FTS, NAG,
                   use_b0, use_b1, use_l0b, use_l1b):
    NCHUNK = int(sum(K_t))
    E_pad = NCHUNK * 128
    SW = E_pad // 16
    TW = HC + 128  # bf16 table row: h | asrc bf16 x4 | pad (256B-mult rows)
    KB = HC // 128
    GPG = NT // NAG          # dst tiles per allgather group
    GR = GPG * 128           # rows per group per core

    nc = bacc.Bacc(dynamic_dma_scratch_size=65536, num_swdge_queues=4)
    P = nc.declare_dram_parameter

    xT = P("xT", [F_IN, NP], BF16, isOutput=False)
    r0h = P("r0h", [F_IN, HC], BF16, isOutput=False)
    r0a = P("r0a", [F_IN, 8], BF16, isOutput=False)
    r1h = P("r1h", [HC, HC], BF16, isOutput=False)
    r1a = P("r1a", [HC, 8], BF16, isOutput=False)
    r2 = P("r2", [HC, FTS], BF16, isOutput=False)
    r3 = P("r3", [FTS, 1], BF16, isOutput=False)
    b0t = P("b0t", [128, HC], F32, isOutput=False)
    b1t = P("b1t", [128, HC], F32, isOutput=False)
    l0bt = P("l0bt", [128, FTS], F32, isOutput=False)
    l1bt = P("l1bt", [128, 1], F32, isOutput=False)
    identb = P("identb", [128, 128], BF16, isOutput=False)
    srcw0 = P("srcw0", [128, SW], I16, isOutput=False)
    srcw1 = P("srcw1", [128, SW], I16, isOutput=False)
    ohb = P("ohb", [128, NCHUNK * 128], BF16, isOutput=False)
    oht = P("oht", [128, NCHUNK * 128], BF16, isOutput=False)
    ae0 = P("ae0", [128, NCHUNK, 4], BF16, isOutput=False)
    ae1 = P("ae1", [128, NCHUNK, 4], BF16, isOutput=False)
    outp = P("out", [NT * 128, 1], F32, isOutput=True)

    with tile.TileContext(nc) as tc:
        with (
            tc.tile_pool(name="const", bufs=1) as const,
            tc.tile_pool(name="stage", bufs=2) as stage,
            tc.tile_pool(name="work", bufs=3) as work,
            tc.tile_pool(name="tp", bufs=6) as tp,
            tc.tile_pool(name="adp", bufs=1) as adp,
            tc.tile_pool(name="psacc", bufs=2, space="PSUM") as psacc,
            tc.tile_pool(name="pss", bufs=2, space="PSUM") as pss,
            tc.tile_pool(name="pstr", bufs=2, space="PSUM") as pstr,
            tc.tile_pool(name="dram", bufs=1, space="DRAM") as dram,
        ):
            TWC = HC + 8  # useful row prefix: h | asrc (1040B)
            H0 = dram.tile([NP, TW], BF16, tag="H0")
            H1 = dram.tile([NP, TW], BF16, tag="H1")
            H1g = [dram.tile([GR, TW], BF16, tag=f"H1g{g}", name=f"H1g{g}")
                   for g in range(NAG)]

            _cn = [0]

            def cload(ap_in, shape, dt=F32, tag=None):
                _cn[0] += 1
                cname = tag or f"c{_cn[0]}"
                t = const.tile(shape, dt, tag=cname, name=f"{cname}_{_cn[0]}")
                nc.sync.dma_start(out=t[:], in_=ap_in)
                return t

            r0h_s = cload(r0h[:, :], [F_IN, HC], BF16)
            r0a_s = cload(r0a[:, :], [F_IN, 8], BF16)
            r1h_s = [cload(r1h[k * 128:(k + 1) * 128, :], [128, HC], BF16)
                     for k in range(KB)]
            r1a_s = [cload(r1a[k * 128:(k + 1) * 128, :], [128, 8], BF16)
                     for k in range(KB)]
            r2_s = [cload(r2[k * 128:(k + 1) * 128, :], [128, FTS], BF16)
                    for k in range(KB)]
            r3_s = cload(r3[:, :], [FTS, 1], BF16)
            id_s = cload(identb[:, :], [128, 128], BF16)
            srcw0_s = cload(srcw0[:, :], [128, SW], I16)
            srcw1_s = cload(srcw1[:, :], [128, SW], I16)
            ae0_s = cload(ae0[:, :, :], [128, NCHUNK, 4], BF16, tag="ae0")
            ae1_s = cload(ae1[:, :, :], [128, NCHUNK, 4], BF16, tag="ae1")
            b0_s = cload(b0t[:, :], [128, HC]) if use_b0 else None
            b1_s = cload(b1t[:, :], [128, HC]) if use_b1 else None
            l0b_s = cload(l0bt[:, :], [128, FTS]) if use_l0b else None
            l1b_s = cload(l1bt[:, :], [128, 1]) if use_l1b else None

            adt0 = {}
            adt1 = {}

            # ---- phase A: layer-0 table (xT comes pre-permuted: own-first)
            MT = NP // 128
            XB = 8  # x tiles per write block
            nc._state.push_named_scope("phaseA")
            for blk in range(MT // XB):
                lx = stage.tile([F_IN, XB * 128], BF16, tag="lx", bufs=2)
                nc.sync.dma_start(
                    out=lx[:], in_=xT[:, blk * XB * 128:(blk + 1) * XB * 128])
                lo = 0
                stw = stage.tile([128, XB, TWC], BF16, tag="hblk", bufs=2)
                for j in range(XB):
                    mr = blk * XB + j
                    lj = lo + j
                    ph = psacc.tile([128, HC], F32, tag="ph")
                    nc.tensor.matmul(ph[:], lx[:, lj * 128:(lj + 1) * 128],
                                     r0h_s[:], start=True, stop=True)
                    pa = pss.tile([128, 8], F32, tag="pss")
                    nc.tensor.matmul(pa[:], lx[:, lj * 128:(lj + 1) * 128],
                                     r0a_s[:], start=True, stop=True)
                    if mr % 2 == 0:
                        nc.vector.tensor_copy(stw[:, j, 0:HC], ph[:])
                    else:
                        nc.scalar.activation(stw[:, j, 0:HC], ph[:], AF.Copy)
                    nc.vector.tensor_copy(stw[:, j, HC:HC + 4], pa[:, 0:4])
                    if mr < NT:
                        ad = adp.tile([128, 4], BF16, tag=f"adt0_{mr}",
                                      name=f"adt0_{mr}")
                        nc.vector.tensor_copy(ad[:], pa[:, 4:8])
                        adt0[mr] = ad
                # one blocked write: H0 rows [blk*XB*128, (blk+1)*XB*128)
                base = blk * XB * 128 * TW
                h0blk = bass.AP(H0.tensor, base,
                                [[TW, 128], [128 * TW, XB], [1, TWC]])
                nc.sync.dma_start(out=h0blk, in_=stw[:])
            nc._state.pop_named_scope("phaseA")

            # ---- aggregation over one layer's edges ----
            def agg_layer(tbl, srcw_s, ae_s, adts, finalize):
                # chunk q -> owning dst slot
                t_of_q = []
                for t in range(NT):
                    t_of_q += [t] * K_t[t]
                NSUP = -(-NCHUNK // SCC)

                def ensure_super(s):
                    cnt = min(SCC * 128, E_pad - s * SCC * 128)
                    nch = cnt // 128
                    gA = stage.tile([128, SCC, TW], BF16, tag="gA",
                                    name="gA", bufs=4)
                    c0 = s * SCC * 8
                    qparts = []
                    base = 0
                    nq = min(4, nch)
                    for qi in range(nq):
                        take = (nch - base + (nq - qi) - 1) // (nq - qi)
                        qparts.append((base, take, qi))
                        base += take
                    for (b0, tk, qi) in qparts:
                        nc.gpsimd.dma_gather(
                            gA[:, b0:b0 + tk, :], tbl[:, :],
                            srcw_s[:, c0 + b0 * 8:c0 + (b0 + tk) * 8],
                            tk * 128, tk * 128, TW,
                            single_packet=False, queue_num=qi)
                    ohb_t = stage.tile([128, SCC * 128], BF16, tag="ohb",
                                       name="ohb_t", bufs=3)
                    nc.sync.dma_start(
                        out=ohb_t[:, 0:nch * 128],
                        in_=ohb[:, s * SCC * 128:s * SCC * 128 + nch * 128])
                    oht_t = stage.tile([128, SCC * 128], BF16, tag="oht",
                                       name="oht_t", bufs=2)
                    nc.sync.dma_start(
                        out=oht_t[:, 0:nch * 128],
                        in_=oht[:, s * SCC * 128:s * SCC * 128 + nch * 128])
                    # expand alpha_dst per edge via ohT matmul; stage to SBUF
                    # so the PSUM bank frees fast and peads can run far ahead
                    pead = pstr.tile([128, SCC * 4], F32, tag="pt", name="pead")
                    for jj in range(nch):
                        qq = s * SCC + jj
                        nc.tensor.matmul(
                            pead[:, jj * 4:(jj + 1) * 4],
                            oht_t[:, jj * 128:(jj + 1) * 128],
                            adts[t_of_q[qq]][:],
                            start=True, stop=True)
                    peads = work.tile([128, SCC * 4], F32, tag="peads", bufs=6)
                    nc.vector.tensor_copy(peads[:, 0:nch * 4],
                                          pead[:, 0:nch * 4])
                    t0 = work.tile([128, SCC, 4], BF16, tag="t0", bufs=2)
                    nc.vector.tensor_add(
                        t0[:, 0:nch, :], gA[:, 0:nch, HC:HC + 4],
                        ae_s[:, s * SCC:s * SCC + nch, :])
                    t1 = work.tile([128, SCC, 4], F32, tag="t1", bufs=2)
                    nc.vector.tensor_add(
                        t1[:, 0:nch, :], t0[:, 0:nch, :],
                        peads[:, 0:nch * 4].rearrange("x (a b) -> x a b", b=4))
                    t2 = work.tile([128, SCC, 4], F32, tag="t2", bufs=2)
                    nc.scalar.activation(
                        t2[:, 0:nch, :], t1[:, 0:nch, :], AF.Copy, scale=0.2)
                    t3 = work.tile([128, SCC, 4], F32, tag="t3", bufs=2)
                    nc.vector.tensor_max(
                        t3[:, 0:nch, :], t1[:, 0:nch, :], t2[:, 0:nch, :])
                    p = work.tile([128, SCC, 4], F32, tag="p", bufs=2)
                    nc.scalar.activation(p[:, 0:nch, :], t3[:, 0:nch, :], AF.Exp)
                    pb = work.tile([128, SCC, 4], BF16, tag="pb", bufs=2)
                    nc.scalar.activation(pb[:, 0:nch, :], p[:, 0:nch, :], AF.Copy)
                    return gA, ohb_t, p, pb

                q = 0
                gA = ohb_t = p = pb = None
                for t in range(NT):
                    ps_o = psacc.tile([128, HC], F32, tag="ph")
                    ps_s = pss.tile([128, 8], F32, tag="pss")
                    for k in range(K_t[t]):
                        s, j = divmod(q, SCC)
                        if j == 0:
                            gA, ohb_t, p, pb = ensure_super(s)
                        gp = work.tile([128, HC], BF16, tag="gp", bufs=4)
                        nc.vector.tensor_mul(
                            gp[:].rearrange("x (h c) -> x h c", h=H),
                            gA[:, j, 0:HC].rearrange("x (h c) -> x h c", h=H),
                            _bcast4(p, j, C))
                        first, last = (k == 0), (k == K_t[t] - 1)
                        oh_j = ohb_t[:, j * 128:(j + 1) * 128]
                        nc.tensor.matmul(ps_o[:], oh_j, gp[:],
                                         start=first, stop=last)
                        nc.tensor.matmul(ps_s[:, 0:4], oh_j, pb[:, j, :],
                                         start=first, stop=last)
                        q += 1
                    finalize(t, ps_o, ps_s)

            def norm_relu(ps_o, ps_s, bias_s):
                sp = work.tile([128, 4], F32, tag="sp")
                nc.vector.tensor_scalar_add(sp[:], ps_s[:, 0:4], 1e-16)
                rc = work.tile([128, 4], F32, tag="rc")
                nc.vector.reciprocal(rc[:], sp[:])
                ar = work.tile([128, HC], BF16, tag="ar", bufs=2)
                if bias_s is None:
                    # relu(x*c) == c*relu(x) for c>0: fused PSUM->bf16
                    for h in range(H):
                        nc.scalar.activation(
                            ar[:, h * C:(h + 1) * C], ps_o[:, h * C:(h + 1) * C],
                            AF.Relu, scale=rc[:, h:h + 1])
                else:
                    ao = work.tile([128, HC], F32, tag="ao", bufs=2)
                    nc.vector.tensor_mul(
                        ao[:].rearrange("x (h c) -> x h c", h=H),
                        ps_o[:].rearrange("x (h c) -> x h c", h=H),
                        _bcast_free(rc, C))
                    ab = work.tile([128, HC], F32, tag="ao", bufs=2)
                    nc.vector.tensor_add(ab[:], ao[:], bias_s[:])
                    nc.scalar.activation(ar[:], ab[:], AF.Relu)
                return ar

            # ---- layer-0 finalize: transpose + layer-1 linear + H1 group AG
            def fin0(t, ps_o, ps_s):
                ar = norm_relu(ps_o, ps_s, b0_s)
                a0k = []
                for kk in range(KB):
                    pt = pstr.tile([128, 128], BF16, tag="ptb")
                    nc.tensor.transpose(pt[:], ar[:, kk * 128:(kk + 1) * 128],
                                        id_s[:])
                    ak = tp.tile([128, 128], BF16, tag="a1T", name=f"a0k{kk}")
                    nc.vector.tensor_copy(ak[:], pt[:])
                    a0k.append(ak)
                ph1 = psacc.tile([128, HC], F32, tag="ph")
                pa1 = pss.tile([128, 8], F32, tag="pss")
                for kk in range(KB):
                    first, last = (kk == 0), (kk == KB - 1)
                    nc.tensor.matmul(ph1[:], a0k[kk][:], r1h_s[kk][:],
                                     start=first, stop=last)
                    nc.tensor.matmul(pa1[:], a0k[kk][:], r1a_s[kk][:],
                                     start=first, stop=last)
                st = stage.tile([128, TWC], BF16, tag="hrow", bufs=2)
                if t % 2 == 0:
                    nc.vector.tensor_copy(st[:, 0:HC], ph1[:])
                else:
                    nc.scalar.activation(st[:, 0:HC], ph1[:], AF.Copy)
                nc.vector.tensor_copy(st[:, HC:HC + 4], pa1[:, 0:4])
                ad = adp.tile([128, 4], BF16, tag=f"adt1_{t}", name=f"adt1_{t}")
                nc.vector.tensor_copy(ad[:], pa1[:, 4:8])
                adt1[t] = ad
                g = t // GPG
                loc = t % GPG
                nc.sync.dma_start(
                    out=H1g[g][loc * 128:(loc + 1) * 128, 0:TWC], in_=st[:])
                if loc == GPG - 1:
                    nc.gpsimd.collective_compute(
                        "AllGather", OP.bypass,
                        replica_groups=[list(range(NCORES))],
                        ins=[H1g[g].opt()],
                        outs=[H1[g * NCORES * GR:(g + 1) * NCORES * GR,
                                 :].opt()],
                    )

            nc._state.push_named_scope("phaseB")
            agg_layer(H0, srcw0_s, ae0_s, adt0, fin0)
            nc._state.pop_named_scope("phaseB")

            # ---- layer-1 aggregation + MLP head per dst tile ----
            def fin1(t, ps_o, ps_s):
                ar = norm_relu(ps_o, ps_s, b1_s)
                h2p = psacc.tile([128, FTS], F32, tag="ph")
                for kk in range(KB):
                    pt = pstr.tile([128, 128], BF16, tag="ptb")
                    nc.tensor.transpose(pt[:], ar[:, kk * 128:(kk + 1) * 128],
                                        id_s[:])
                    a1k = tp.tile([128, 128], BF16, tag="a1T")
                    nc.vector.tensor_copy(a1k[:], pt[:])
                    nc.tensor.matmul(h2p[:], a1k[:], r2_s[kk][:],
                                     start=(kk == 0), stop=(kk == KB - 1))
                if use_l0b:
                    h2b = work.tile([128, FTS], F32, tag="h2b")
                    nc.vector.tensor_add(h2b[:], h2p[:], l0b_s[:])
                else:
                    h2b = h2p
                h2r = work.tile([128, FTS], BF16, tag="h2r")
                nc.scalar.activation(h2r[:], h2b[:], AF.Relu)
                pt2 = pstr.tile([128, 128], BF16, tag="ptb")
                nc.tensor.transpose(pt2[:], h2r[:], id_s[:])
                h2T = tp.tile([128, 128], BF16, tag="a1T")
                nc.vector.tensor_copy(h2T[:], pt2[:])
                po = pss.tile([128, 8], F32, tag="pss")
                nc.tensor.matmul(po[:, 0:1], h2T[:], r3_s[:],
                                 start=True, stop=True)
                ob = work.tile([128, 1], F32, tag="ob")
                if use_l1b:
                    nc.vector.tensor_add(ob[:], po[:, 0:1], l1b_s[:])
                else:
                    nc.vector.tensor_copy(ob[:], po[:, 0:1])
                nc.sync.dma_start(out=outp[t * 128:(t + 1) * 128, :],
                                  in_=ob[:])

            nc._state.push_named_scope("phaseD")
            agg_layer(H1, srcw1_s, ae1_s, adt1, fin1)
            nc._state.pop_named_scope("phaseD")

    nc.finalize()
    return nc


def _wrap_idx(v, E_pad):
    blk = np.zeros((16, E_pad // 16), np.int16)
    ar = np.arange(E_pad)
    blk[ar % 16, ar // 16] = v.astype(np.int16)
    return np.tile(blk, (8, 1))


def kernel(x, edge_index, edge_weights,
           W0, as0, ad0, We0, ae0, b0,
           W1, as1, ad1, We1, ae1, b1,
           L0W, L0b, L1W, L1b):
    x = np.asarray(x, np.float32)
    N, F_IN = x.shape
    HC = W0.shape[0]
    H, C = np.asarray(as0).shape
    FTS = np.asarray(L0W).shape[0]

    NT = -(-N // (128 * NCORES))
    SHARD = NT * 128
    NP = SHARD * NCORES
    NAG = 1
    for cand in (10, 5, 4, 2):
        if NT % cand == 0:
            NAG = cand
            break
    GPG = NT // NAG
    GR = GPG * 128

    # ---- edges ----
    ew_in = np.asarray(edge_weights, np.float32)
    src = np.concatenate([np.asarray(edge_index[0]), np.arange(N)])
    dst = np.concatenate([np.asarray(edge_index[1]), np.arange(N)])
    ew = np.concatenate([ew_in, np.full(N, ew_in.mean(), np.float32)])
    order = np.argsort(dst, kind="stable")
    src_s, dst_s, ew_s = src[order], dst[order], ew[order]

    NTG = NP // 128
    tile_of = (dst_s // 128).astype(np.int64)
    tcounts = np.bincount(tile_of, minlength=NTG)
    tstart = np.concatenate([[0], np.cumsum(tcounts)])

    # ---- slot load balancing: slot r = each core's r-th largest tile ----
    cmat = tcounts.reshape(NCORES, NT)
    perm = np.argsort(-cmat, axis=1, kind="stable")   # [core, slot] -> tile
    slot_of = np.empty_like(perm)
    for i in range(NCORES):
        slot_of[i, perm[i]] = np.arange(NT)
    K_t = [max(1, int(max(-(-cmat[i, perm[i, r]] // 128)
                          for i in range(NCORES))))
           for r in range(NT)]
    NCHUNK = int(sum(K_t))
    E_pad = NCHUNK * 128

    # ---- node -> H1 table row (AllGather layout: group, core, loc) ----
    nodes = np.arange(NP)
    core = nodes // SHARD
    rr = nodes % SHARD
    tloc = rr // 128
    off = rr % 128
    slot_n = slot_of[core, tloc]
    gg = slot_n // GPG
    loc = slot_n % GPG
    t1row_of_n = gg * (NCORES * GR) + core * GR + loc * 128 + off

    # ---- weight folding (host, O(weights)) ----
    as0 = np.asarray(as0, np.float32)
    ad0 = np.asarray(ad0, np.float32)
    ae0w = np.asarray(ae0, np.float32)
    as1 = np.asarray(as1, np.float32)
    ad1 = np.asarray(ad1, np.float32)
    ae1w = np.asarray(ae1, np.float32)
    W0 = np.asarray(W0, np.float32)
    W1 = np.asarray(W1, np.float32)
    We0 = np.asarray(We0, np.float32)
    We1 = np.asarray(We1, np.float32)

    k0 = (We0.reshape(H, C) * ae0w).sum(1).astype(np.float32)
    k1 = (We1.reshape(H, C) * ae1w).sum(1).astype(np.float32)

    def fold(W, a):
        blk = np.zeros((HC, H), np.float32)
        for h in range(H):
            blk[h * C:(h + 1) * C, h] = a[h]
        return (W.T @ blk).astype(np.float32)

    bf = ml_dtypes.bfloat16
    r0h = W0.T.astype(bf)
    r0a = np.concatenate([fold(W0, as0), fold(W0, ad0)], 1).astype(bf)
    r1h = W1.T.astype(bf)
    r1a = np.concatenate([fold(W1, as1), fold(W1, ad1)], 1).astype(bf)
    r2 = np.asarray(L0W, np.float32).T.astype(bf)
    r3 = np.asarray(L1W, np.float32).T.astype(bf)

    b0t = np.tile(np.asarray(b0, np.float32)[None, :], (128, 1))
    b1t = np.tile(np.asarray(b1, np.float32)[None, :], (128, 1))
    l0bt = np.tile(np.asarray(L0b, np.float32)[None, :], (128, 1))
    l1bt = np.tile(np.asarray(L1b, np.float32).reshape(1, 1), (128, 1))
    identb = np.eye(128, dtype=np.float32).astype(bf)

    xa = np.zeros((NP, F_IN), np.float32)
    xa[:N] = x

    in_maps = []
    for i in range(NCORES):
        # per-core own-first H0 row layout
        own = np.empty(SHARD, np.int64)        # H0 row r*128+o -> node
        for r in range(NT):
            t = perm[i, r]
            own[r * 128:(r + 1) * 128] = i * SHARD + t * 128 + np.arange(128)
        is_own = core == i
        foreign = nodes[~is_own][np.argsort(t1row_of_n[~is_own], kind="stable")]
        row0_inv = np.concatenate([own, foreign])   # H0 row -> node
        row0_of = np.empty(NP, np.int64)
        row0_of[row0_inv] = nodes                   # node -> H0 row

        xT0 = np.ascontiguousarray(xa[row0_inv].T).astype(bf)

        src0 = np.zeros(E_pad, np.int64)
        src1 = np.zeros(E_pad, np.int64)
        dlocp = np.full(E_pad, -1, np.int64)
        ewp = np.zeros(E_pad, np.float32)
        offq = 0
        for r in range(NT):
            t = perm[i, r]
            g = i * NT + t
            cnt = int(tcounts[g])
            sl = slice(tstart[g], tstart[g] + cnt)
            src0[offq:offq + cnt] = row0_of[src_s[sl]]
            src1[offq:offq + cnt] = t1row_of_n[src_s[sl]]
            dlocp[offq:offq + cnt] = dst_s[sl] - (i * SHARD + t * 128)
            ewp[offq:offq + cnt] = ew_s[sl]
            # sort each 128-edge chunk by src row for gather locality
            for cb in range(offq, offq + cnt, 128):
                ce = min(cb + 128, offq + cnt)
                so = np.argsort(src1[cb:ce], kind="stable")
                src0[cb:ce] = src0[cb:ce][so]
                src1[cb:ce] = src1[cb:ce][so]
                dlocp[cb:ce] = dlocp[cb:ce][so]
                ewp[cb:ce] = ewp[cb:ce][so]
            offq += K_t[r] * 128
        ae0p = (ewp[:, None] * k0[None, :]).reshape(NCHUNK, 128, 4)
        ae0p = ae0p.transpose(1, 0, 2)
        ae1p = (ewp[:, None] * k1[None, :]).reshape(NCHUNK, 128, 4)
        ae1p = ae1p.transpose(1, 0, 2)
        # one-hot blocks: ohb[e, q*128 + d] / oht[d, q*128 + e]
        ohcube = np.zeros((NCHUNK, 128, 128), np.float32)  # [q, e, d]
        dl2 = dlocp.reshape(NCHUNK, 128)
        valid = dl2 >= 0
        qs, es = np.nonzero(valid)
        ohcube[qs, es, dl2[qs, es]] = 1.0
        ohb_np = np.ascontiguousarray(
            ohcube.transpose(1, 0, 2).reshape(128, NCHUNK * 128)).astype(bf)
        oht_np = np.ascontiguousarray(
            ohcube.transpose(2, 0, 1).reshape(128, NCHUNK * 128)).astype(bf)
        in_maps.append({
            "xT": xT0, "r0h": r0h, "r0a": r0a, "r1h": r1h, "r1a": r1a,
            "r2": r2, "r3": r3, "b0t": b0t, "b1t": b1t, "l0bt": l0bt,
            "l1bt": l1bt, "identb": identb,
            "srcw0": _wrap_idx(src0, E_pad), "srcw1": _wrap_idx(src1, E_pad),
            "ohb": ohb_np, "oht": oht_np,
            "ae0": np.ascontiguousarray(ae0p).astype(bf),
            "ae1": np.ascontiguousarray(ae1p).astype(bf),
        })

    nc = _build_program(NP, F_IN, HC, H, C, NT, K_t, FTS, NAG,
                        bool(np.any(b0)), bool(np.any(b1)),
                        bool(np.any(np.asarray(L0b))),
                        bool(np.any(np.asarray(L1b))))
    res = run_bass_kernel_spmd(nc, in_maps, list(range(NCORES)))
    out = np.empty(NP, np.float32)
    for i in range(NCORES):
        oi = res.results[i]["out"][:, 0]
        for t in range(NT):
            r = slot_of[i, t]
            out[i * SHARD + t * 128:i * SHARD + (t + 1) * 128] = \
                oi[r * 128:(r + 1) * 128]
    return out[:N].astype(np.float32)
